# revision 1
# baseline (speedup 1.0000x reference)
"""DualAttention Trainium2 kernel (v2: fp8 DoubleRow attention path).

Sharding: 8 cores = 4 samples x 2 query-halves. Per core the sample image is
"rolled" by the half offset (host-side, with correct zero padding), so every
core runs the identical program on its first 2048 query positions; attention
over key positions is permutation-invariant, so convs/attention on the rolled
image give the true result for the core's half.

Precision plan (validated vs reference in numpy, rel err ~8e-3):
  qk conv + scores: fp32r (score noise is the binding error term).
  v conv: fp8e4m3 DoubleRow (x and 64*wv quantized to fp8; contract 2x128
    per pass -> 4x fewer PE cycles than fp32r).
  v transposed via DMA xbar transpose (bf16), quantized to fp8 (8*v).
  exp -> fp8 E tiles directly (exp(s-3), range safe for e4m3).
  U = vt8 @ E8 and denominator = ones8 @ E8 both fp8 DoubleRow; the ones
    matmul broadcasts the denominator across all 128 partitions for free.
  local = U * (1/denom) on DVE from PSUM; 1/8 v-scale folded into fuse w.
  SE/glob/fuse: fp32(r) as before.
"""

import sys

sys.path.insert(0, "/opt/trn_rl_repo")

import numpy as np
import ml_dtypes

import concourse.bass as bass
import concourse.mybir as mybir
import concourse.tile as tile
from concourse import bacc
from concourse.bass_utils import run_bass_kernel_spmd

f32 = mybir.dt.float32
f32r = mybir.dt.float32r
fp8 = mybir.dt.float8e4
bf16 = mybir.dt.bfloat16
AF = mybir.ActivationFunctionType
DR = mybir.MatmulPerfMode.DoubleRow
e4m3 = ml_dtypes.float8_e4m3fn

C = 256
CT = 2          # channel tiles of 128
Cr = 64
H = W = 64
HW = H * W      # 4096
HWh = 2048      # query positions per core
JT = 32         # key-position tiles of 128
IH = 2          # i halves of 1024
ICH = 2         # 512-chunks per i half
N_CORES = 8
EXPB = 3.0      # exp bias: E = exp(s - EXPB), cancels in softmax

_compiled = None


def _build(debug=False, parts=("qk", "vt8", "loc", "glob", "et", "recb", "usb")):
    nc = bacc.Bacc("TRN2", target_bir_lowering=False, debug=False,
                   num_devices=N_CORES)
    parts = set(parts) if debug else set()

    xp_d = nc.declare_dram_parameter("xp", [C, 68 * 66], f32r, isOutput=False)
    xp8_d = nc.declare_dram_parameter("xp8", [128, 2 * 68 * 66], fp8, isOutput=False)
    wqkt_d = nc.declare_dram_parameter("wqkt", [18, 128, 128], f32r, isOutput=False)
    wv8_d = nc.declare_dram_parameter("wv8", [9, 128, 512], fp8, isOutput=False)
    smallp_d = nc.declare_dram_parameter("smallp", [128, 37], f32, isOutput=False)
    smallq_d = nc.declare_dram_parameter("smallq", [16, 257], f32, isOutput=False)
    fuset_d = nc.declare_dram_parameter("fuset", [4, 2, 128, 128], f32r, isOutput=False)
    out_d = nc.declare_dram_parameter("out", [2, 128, HWh], f32, isOutput=True)
    if "qkpin" in parts:
        qk_dbg = nc.declare_dram_parameter("qk_dbg", [128, 64], f32, isOutput=True)
    elif "qk" in parts:
        qk_dbg = nc.declare_dram_parameter("qk_dbg", [128, HW], f32, isOutput=True)
    if "vt8" in parts:
        vt8_dbg = nc.declare_dram_parameter("vt8_dbg", [128, 16 * 512], fp8, isOutput=True)
    if "loc" in parts:
        loc_dbg = nc.declare_dram_parameter("loc_dbg", [2, 128, HWh], f32, isOutput=True)
    if "et" in parts:
        et_dbg = nc.declare_dram_parameter("et_dbg", [16, 128, 2048], fp8, isOutput=True)
    if "recb" in parts:
        recb_dbg = nc.declare_dram_parameter("recb_dbg", [2, 128, 1024], f32, isOutput=True)
    if "usb" in parts:
        usb_dbg = nc.declare_dram_parameter("usb_dbg", [2, 128, HWh], f32, isOutput=True)
    if "glob" in parts:
        glob_dbg = nc.declare_dram_parameter("glob_dbg", [2, 128, HWh], f32, isOutput=True)

    with tile.TileContext(nc) as tc, \
         nc.allow_low_precision(reason="fp8/f32r storage; validated numerics"):
      with tc.tile_pool(name="pw", bufs=1) as pw, \
           tc.tile_pool(name="pxv", bufs=1) as pxv:
        # persistent tiles
        wqkt = pw.tile([128, 18, 128], f32r)
        w8v = pw.tile([128, 9, 2, 256], fp8)
        fuset = pw.tile([128, 8, 128], f32r)
        sp = pw.tile([128, 37], f32)
        sq = pw.tile([16, 257], f32)

        qk = pw.tile([128, HW], f32r)         # q rows 0-63 (cols 0-2047), k rows 64-127
        q64 = pw.tile([128, HWh], f32r)       # q copy at partitions 64-127
        klo = pw.tile([64, HW], f32r)         # k copy at partitions 0-63
        vt8 = pw.tile([128, 16, 2, 256], fp8)  # v^T, fp8(8*v), pair-slotted
        glob = [pw.tile([128, 32, 64], f32r, tag=f"glob{t}", name=f"glob{t}") for t in range(CT)]
        yse = [pw.tile([128, 1], f32, tag=f"yse{t}", name=f"yse{t}") for t in range(CT)]
        loc = [pw.tile([128, HWh], f32r, tag=f"loc{t}", name=f"loc{t}") for t in range(CT)]
        recb = pw.tile([128, 1024], f32)
        ones8 = pw.tile([128, 2, 128], fp8)
        bneg = pw.tile([128, 1], f32)
        usb = [pw.tile([128, HWh], f32, tag=f"usb{t}", name=f"usb{t}")
               for t in range(CT)]

        with tc.tile_pool(name="px", bufs=1) as px, \
             tc.tile_pool(name="psqk", bufs=4, space="PSUM") as psqk, \
             tc.tile_pool(name="psv", bufs=3, space="PSUM") as psv, \
             tc.tile_pool(name="psse", bufs=1, space="PSUM") as psse:
            xp = [px.tile([128, 68, 66], f32r, tag=f"xp{j}", name=f"xp{j}") for j in range(CT)]
            xp8 = pxv.tile([128, 2, 68, 66], fp8)
            v_sb = [pxv.tile([128, HW], bf16, tag=f"vsb{t}", name=f"vsb{t}")
                    for t in range(CT)]
            vt_b = pxv.tile([128, 32, 256], bf16)

            nc.sync.dma_start(wqkt[:, 0:6:2, :],
                              wqkt_d[0:6:2].rearrange("t p m -> p t m"))
            nc.sync.dma_start(wqkt[:, 6:18:2, :],
                              wqkt_d[6:18:2].rearrange("t p m -> p t m"))
            for j in range(CT):
                src = xp_d[j * 128:(j + 1) * 128, :].rearrange(
                    "p (h w) -> p h w", w=66)
                nc.gpsimd.dma_start(xp[j][:, 0:6, :], src[:, 0:6, :])
            for j in range(CT):
                src = xp_d[j * 128:(j + 1) * 128, :].rearrange(
                    "p (h w) -> p h w", w=66)
                nc.gpsimd.dma_start(xp[j][:, 6:12, :], src[:, 6:12, :])
            nc.sync.dma_start(wqkt[:, 1:18:2, :],
                              wqkt_d[1:18:2].rearrange("t p m -> p t m"))
            nc.sync.dma_start(sp[:], smallp_d[:])
            nc.sync.dma_start(sq[:], smallq_d[:])
            for j in range(CT):
                src = xp_d[j * 128:(j + 1) * 128, :].rearrange(
                    "p (h w) -> p h w", w=66)
                for r0, r1 in [(12, 23), (23, 34),
                               (34, 46), (46, 57), (57, 68)]:
                    nc.sync.dma_start(xp[j][:, r0:r1, :], src[:, r0:r1, :])
            # fp8 image + v weights (needed only once the v conv starts)
            nc.sync.dma_start(
                xp8[:].rearrange("p a b c -> p (a b c)"), xp8_d[:])
            nc.sync.dma_start(
                w8v[:].rearrange("p t a b -> p t (a b)"),
                wv8_d[:].rearrange("t p m -> p t m"))
            nc.sync.dma_start(
                fuset[:].rearrange("p (k m) f -> p k m f", k=4),
                fuset_d[:].rearrange("k m p f -> p k m f"),
            )

            # constants
            onesf = px.tile([128, 2, 128], f32)
            nc.vector.memset(onesf[:], 1.0)
            nc.vector.tensor_copy(ones8[:], onesf[:])
            nc.vector.memset(bneg[:], -EXPB)

            # ---- SE channel sums (mean folded into fc1 weights host-side)
            sums = [pw.tile([128, 1], f32, tag=f"sums{j}", name=f"sums{j}") for j in range(CT)]
            sa = pw.tile([128, 1], f32)
            sb_ = pw.tile([128, 1], f32)
            for j in range(CT):
                nc.vector.reduce_sum(sa[:], xp[j][:, 1:33, 1:65].bitcast(f32),
                                     axis=mybir.AxisListType.XY)
                nc.vector.reduce_sum(sb_[:], xp[j][:, 35:67, 1:65].bitcast(f32),
                                     axis=mybir.AxisListType.XY)
                nc.vector.tensor_add(sums[j][:], sa[:], sb_[:])

            # ---- SE MLP: y = sigmoid(fc2 @ relu(fc1 @ mean + b1) + b2)
            ps1 = psse.tile([16, 1], f32)
            for j in range(CT):
                nc.tensor.matmul(ps1[:], sp[:, 5 + j * 16:5 + (j + 1) * 16], sums[j][:],
                                 start=(j == 0), stop=(j == CT - 1))
            y1 = pw.tile([16, 1], f32)
            nc.scalar.activation(y1[:], ps1[:], AF.Relu, bias=sq[0:16, 256:257])
            for t in range(CT):
                ps2 = psse.tile([128, 1], f32, tag="ps1", name="ps2")
                nc.tensor.matmul(ps2[:], sq[0:16, t * 128:(t + 1) * 128], y1[:],
                                 start=True, stop=True)
                # sigmoid(z) = 1/(1+exp(-z)), z = ps2 + fc2b ; fc2bn = -fc2b
                en = pw.tile([128, 1], f32, tag="en")
                nc.scalar.activation(en[:], ps2[:], AF.Exp,
                                     bias=sp[:, 3 + t:4 + t], scale=-1.0)
                nc.vector.tensor_scalar_add(en[:], en[:], 1.0)
                nc.vector.reciprocal(yse[t][:], en[:])

            # ---- fused q+k conv (co=128, full rolled image, fp32r);
            # chunk 0 is split in two 4-row halves so the very first
            # matmul only needs xp rows 0:6 from the head DMA
            qk_chunks = [(0, 0, 4), (0, 4, 4)] + [
                (c, c * 8 if c < 4 else 34 + (c - 4) * 8, 8)
                for c in range(1, 8)]
            for c, base, nrow in qk_chunks:
                pqk = psqk.tile([128, nrow * 64], f32, name="pqk")
                first = True
                for j in range(CT):
                    for dy in range(3):
                        for dx in range(3):
                            t = (dy * 3 + dx) * 2 + j
                            nc.tensor.matmul(
                                pqk[:],
                                wqkt[:, t, :],
                                xp[j][:, base + dy:base + dy + nrow, dx:dx + 64],
                                start=first,
                                stop=(t == 17),
                            )
                            first = False
                o0 = base * 64 if c == 0 else c * 512
                nc.vector.tensor_scalar_add(
                    qk[:, o0:o0 + nrow * 64], pqk[:], sp[:, 0:1])
                csl = slice(o0, o0 + nrow * 64)
                nc.gpsimd.dma_start(klo[:, csl], qk[64:128, csl])
                if c < 4:
                    nc.gpsimd.dma_start(q64[64:128, csl], qk[0:64, csl])

            # ---- glob = x_half * yse  (before xp pool closes)
            for t in range(CT):
                nc.vector.tensor_scalar_mul(glob[t][:],
                                            xp[t][:, 1:33, 1:65].bitcast(f32),
                                            yse[t][:, 0:1])

        # ---- attention ----
        with tc.tile_pool(name="pet", bufs=18) as pet, \
             tc.tile_pool(name="psT", bufs=2, space="PSUM") as psT:

            def emit_A(ih, psv, et_half):
                # scores+exp steps; during half 0 one v-conv quantum is
                # woven into each jt step (fills the ACT-bound gaps on PE);
                # a generator so a prefix of half-1 steps can be emitted
                # before half-0's U chains (ACT overlap across the B phase)
                i0 = ih * 1024
                for jt in range(JT):
                    pT = psT.tile([128, 1024], f32, tag="pT", name="pT")
                    for icq in range(ICH):
                        isl = slice(i0 + icq * 512, i0 + (icq + 1) * 512)
                        psl = pT[:, icq * 512:(icq + 1) * 512]
                        if jt % 2 == 0:
                            nc.tensor.matmul(psl,
                                             klo[:, jt * 128:(jt + 1) * 128],
                                             qk[0:64, isl],
                                             start=True, stop=True)
                        else:
                            nc.tensor.matmul(psl,
                                             qk[64:128, jt * 128:(jt + 1) * 128],
                                             q64[64:128, isl],
                                             start=True, stop=True)
                    if jt % 2 == 0:
                        et = pet.tile([128, 2, 1024], fp8, tag="et", name="et")
                        et_half.append(et)
                    nc.scalar.activation(et[:, jt % 2, :], pT[:], AF.Exp,
                                         bias=bneg[:, 0:1])
                    if psv is not None:
                        ct, ch = jt // 16, jt % 16
                        base = ch * 4 if ch < 8 else 34 + (ch - 8) * 4
                        pv = psv.tile([128, 256], f32, tag="pv", name="pv")
                        for dy in range(3):
                            for dx in range(3):
                                t = dy * 3 + dx
                                nc.tensor.matmul(
                                    pv[:],
                                    w8v[:, t, :, ct * 128:(ct + 1) * 128],
                                    xp8[:, :, base + dy:base + dy + 4,
                                        dx:dx + 64],
                                    start=(t == 0), stop=(t == 8), perf_mode=DR)
                        nc.vector.tensor_scalar_mul(
                            v_sb[ct][:, ch * 256:(ch + 1) * 256], pv[:], 0.125)
                        for vjt in (2 * ch, 2 * ch + 1):
                            nc.sync.dma_start_transpose(
                                vt_b[:, vjt, ct * 128:(ct + 1) * 128],
                                v_sb[ct][:, vjt * 128:(vjt + 1) * 128])
                        if ct == 1:
                            # both ct chunks of pair `ch` transposed: stage
                            # the fp8 pair copy now so B(h0) is not gated on
                            # a serial DVE burst
                            nc.vector.tensor_copy(
                                vt8[:, ch, :, :],
                                vt_b[:, 2 * ch:2 * ch + 2, :])
                    yield jt

            et_halves = {0: [], 1: []}
            with tc.tile_pool(name="psv", bufs=2, space="PSUM") as psv:
                for _ in emit_A(0, psv, et_halves[0]):
                    pass

            with tc.tile_pool(name="psU", bufs=1, space="PSUM") as psU:
              gen1 = emit_A(1, None, et_halves[1])
              for _ in range(4):
                  next(gen1)
              for ih in range(IH):
                i0 = ih * 1024
                et_half = et_halves[ih]
                pu = [psU.tile([128, 1024], f32, tag=f"pu{t}", name=f"pu{t}")
                      for t in range(CT)]
                # U chains run back-to-back AFTER all exps of the half (the
                # interleaved-with-scores form loses accumulation updates on
                # hardware; the contiguous form, like the denominator chain,
                # is exact). PSUM zero regions are whole 2KB banks: only the
                # first 256-chunk of each bank sets start.
                for t in range(CT):
                    for p in range(16):
                        for icq in range(4):
                            nc.tensor.matmul(
                                pu[t][:, icq * 256:(icq + 1) * 256],
                                vt8[:, p, :, t * 128:(t + 1) * 128],
                                et_half[p][:, :, icq * 256:(icq + 1) * 256],
                                start=(p == 0 and icq % 2 == 0),
                                stop=(p == 15),
                                perf_mode=DR, skip_group_check=True)

                # denominator, broadcast across partitions by the ones matmul
                pDB = psT.tile([128, 1024], f32, tag="pT", name="pDB")
                for p in range(16):
                    for icq in range(4):
                        nc.tensor.matmul(
                            pDB[:, icq * 256:(icq + 1) * 256],
                            ones8[:],
                            et_half[p][:, :, icq * 256:(icq + 1) * 256],
                            start=(p == 0 and icq % 2 == 0),
                            stop=(p == 15), perf_mode=DR,
                            skip_group_check=True)
                if "et" in parts and ih == 0:
                    for p in range(16):
                        nc.sync.dma_start(
                            et_dbg[p],
                            et_half[p][:].rearrange("p a b -> p (a b)"))
                nc.vector.reciprocal(recb[:], pDB[:])
                # stage pu in SBUF before multiplying (tensor_copy/activation
                # reads of PSUM are ordered correctly vs later bank reuse;
                # direct DVE tensor_tensor reads of PSUM race on hardware)
                nc.vector.tensor_copy(usb[0][:, i0:i0 + 1024], pu[0][:])
                nc.scalar.activation(usb[1][:, i0:i0 + 1024], pu[1][:], AF.Copy)
                for t in range(CT):
                    nc.vector.tensor_mul(loc[t][:, i0:i0 + 1024],
                                         usb[t][:, i0:i0 + 1024], recb[:])
                if "recb" in parts:
                    nc.sync.dma_start(recb_dbg[ih], recb[:])
                if "usb" in parts:
                    for t in range(CT):
                        nc.sync.dma_start(usb_dbg[t, :, i0:i0 + 1024],
                                          usb[t][:, i0:i0 + 1024])
                if ih == 0:
                    for _ in gen1:
                        pass

        # ---- fuse 1x1 conv + bias (bv and fuse_b folded host-side)
        with tc.tile_pool(name="po", bufs=3) as po, \
             tc.tile_pool(name="psF", bufs=4, space="PSUM") as psF:
            for half in range(2):
                for mt in range(CT):
                    ob = po.tile([128, 1024], f32, tag="ob", name="ob")
                    for sub in range(2):
                        icq = half * 2 + sub
                        isl = slice(icq * 512, (icq + 1) * 512)
                        pf = psF.tile([128, 512], f32, tag="pf", name="pf")
                        for kt in range(4):
                            rhs = (loc[kt][:, isl] if kt < 2 else
                                   glob[kt - 2][:, icq * 8:(icq + 1) * 8, :])
                            nc.tensor.matmul(pf[:], fuset[:, kt * 2 + mt, :],
                                             rhs, start=(kt == 0), stop=(kt == 3))
                        if sub == 0:
                            nc.vector.tensor_scalar_add(
                                ob[:, sub * 512:(sub + 1) * 512], pf[:],
                                sp[:, 1 + mt:2 + mt])
                        else:
                            nc.scalar.activation(
                                ob[:, sub * 512:(sub + 1) * 512], pf[:],
                                AF.Identity, bias=sp[:, 1 + mt:2 + mt])
                        if half == 1:
                            nc.sync.dma_start(out_d[mt, :, isl],
                                              ob[:, sub * 512:(sub + 1) * 512])
                    if half == 0:
                        nc.sync.dma_start(
                            out_d[mt, :, half * 1024:(half + 1) * 1024], ob[:])

            if "qkpin" in parts:
                nc.sync.dma_start(qk_dbg[:], qk[:, 0:64].bitcast(f32))
            elif "qk" in parts:
                nc.sync.dma_start(qk_dbg[:], qk[:].bitcast(f32))
            if "vt8" in parts:
                nc.sync.dma_start(
                    vt8_dbg[:],
                    vt8[:].rearrange("p a b c -> p (a b c)"))
            for t in range(CT):
                if "loc" in parts:
                    nc.sync.dma_start(loc_dbg[t], loc[t][:].bitcast(f32))
                if "glob" in parts:
                    nc.sync.dma_start(
                        glob_dbg[t],
                        glob[t][:].rearrange("p a b -> p (a b)").bitcast(f32))

    nc.compile()
    return nc


def _prep_core_inputs(inputs):
    x = np.ascontiguousarray(inputs["x"], np.float32)
    wq = np.asarray(inputs["wq"], np.float32)
    bq = np.asarray(inputs["bq"], np.float32)
    wk = np.asarray(inputs["wk"], np.float32)
    bk = np.asarray(inputs["bk"], np.float32)
    wv = np.asarray(inputs["wv"], np.float32)
    bv = np.asarray(inputs["bv"], np.float32)
    fc1_w = np.asarray(inputs["fc1_w"], np.float32)
    fc1_b = np.asarray(inputs["fc1_b"], np.float32)
    fc2_w = np.asarray(inputs["fc2_w"], np.float32)
    fc2_b = np.asarray(inputs["fc2_b"], np.float32)
    fuse_w = np.asarray(inputs["fuse_w"], np.float32)[:, :, 0, 0]
    fuse_b = np.asarray(inputs["fuse_b"], np.float32)

    scale = np.float32(Cr ** -0.5)
    wqk = np.concatenate([wq * scale, wk], axis=0)          # [128, 256, 3, 3]
    bqk = np.concatenate([bq * scale, bk])[:, None].astype(np.float32)

    wqkt = np.empty((18, 128, 128), np.float32)
    for dy in range(3):
        for dx in range(3):
            for j in range(CT):
                t = (dy * 3 + dx) * 2 + j
                wqkt[t] = wqk[:, j * 128:(j + 1) * 128, dy, dx].T

    # fp8 v weights (x64), layout [tap, ci_lo(128), ci_tile(2)*co(256)]
    wv8 = np.empty((9, 128, 512), np.float32)
    for dy in range(3):
        for dx in range(3):
            t = dy * 3 + dx
            for tci in range(CT):
                # [co, ci128] -> [ci128, co]
                wv8[t, :, tci * 256:(tci + 1) * 256] = \
                    (64.0 * wv[:, tci * 128:(tci + 1) * 128, dy, dx]).T
    wv8 = np.clip(wv8, -448, 448).astype(e4m3)

    # fuse: local half carries 1/8 (v stored as 8*v)
    fuse_b_eff = fuse_b + fuse_w[:, :C] @ bv
    fuset = np.empty((4, 2, 128, 128), np.float32)
    for kt in range(4):
        s = 0.125 if kt < 2 else 1.0
        for mt in range(CT):
            fuset[kt, mt] = (s * fuse_w[mt * 128:(mt + 1) * 128,
                                        kt * 128:(kt + 1) * 128]).T

    smallp = np.zeros((128, 37), np.float32)
    smallp[:, 0:1] = bqk
    smallp[:, 1:3] = np.stack([fuse_b_eff[t * 128:(t + 1) * 128] for t in range(CT)], axis=1)
    smallp[:, 3:5] = np.stack([-fc2_b[t * 128:(t + 1) * 128] for t in range(CT)], axis=1)
    for j in range(CT):
        smallp[:, 5 + j * 16:5 + (j + 1) * 16] = (fc1_w / HW)[:, j * 128:(j + 1) * 128].T
    smallq = np.zeros((16, 257), np.float32)
    for t in range(CT):
        smallq[:, t * 128:(t + 1) * 128] = fc2_w[t * 128:(t + 1) * 128, :].T
    smallq[:, 256] = fc1_b
    shared = dict(
        wqkt=wqkt, wv8=wv8.view(np.uint8),
        fuset=fuset, smallp=smallp, smallq=smallq,
    )

    in_maps = []
    for core in range(N_CORES):
        s, p = divmod(core, 2)
        s0 = p * 32
        t0 = (s0 + 32) % 64
        P = np.zeros((C, 66, 66), np.float32)
        P[:, 1:65, 1:65] = x[s]
        xp = np.concatenate([P[:, s0:s0 + 34], P[:, t0:t0 + 34]], axis=1)
        m = dict(shared)
        m["xp"] = np.ascontiguousarray(xp.reshape(C, 68 * 66))
        xp8 = np.clip(xp, -448, 448).astype(e4m3)  # [256, 68, 66]
        m["xp8"] = np.ascontiguousarray(
            xp8.reshape(2, 128, 68 * 66).transpose(1, 0, 2).reshape(128, -1)
        ).view(np.uint8)
        in_maps.append(m)
    return in_maps


def kernel(**inputs):
    global _compiled
    if _compiled is None:
        # parts=("qkpin",) keeps one trailing debug DMA of a qk slice: it
        # pins qk's liveness to the end of the program, which shifts tile
        # buffer assignment such that the schedule is correct on hardware
        # (without it, a buffer-reuse race corrupts the local-attention
        # accumulator).
        _compiled = _build(debug=True, parts=("qkpin",))
    nc = _compiled
    in_maps = _prep_core_inputs(inputs)
    res = run_bass_kernel_spmd(nc, in_maps, list(range(N_CORES)))
    out = np.empty((4, C, H, W), np.float32)
    for core in range(N_CORES):
        s, p = divmod(core, 2)
        o = res.results[core]["out"]          # [2, 128, 2048]
        out[s, :, p * 32:(p + 1) * 32, :] = o.reshape(C, 32, 64)
    return out

